# revision 1
# baseline (speedup 1.0000x reference)
"""Bass/Trainium2 kernel for nn_DCDicl (DSBlock forward).

Per sample: Q = Unfold_pad4(x)^T @ Unfold_pad4(x) (+ a*I), P = U^T Yz (+ a*d),
D = cho_solve(Q, P).  The dominant FLOPs (the 25.6 GFLOP/sample Gram matrix)
run on 8 NeuronCores: data-parallel over the 4 samples x 2 halves of the
10000-row contraction.  Host does the unfold layout, the tiny P (64 MFLOP),
and the 1600x1600 solve.
"""

import sys

import numpy as np

if "/opt/trn_rl_repo" not in sys.path:
    sys.path.append("/opt/trn_rl_repo")

N, C_IN, C_OUT, H, W, DS = 4, 64, 4, 96, 96, 5
K = C_IN * DS * DS            # 1600
KP = 1664                     # 13 * 128, padded column count
ROWS = 100 * 100              # unfold output positions
HALF = 5120                   # 40 * 128 rows per core (2 halves of 10000, padded)
KCH = HALF // 128             # 40 k-chunks
NT = 256                      # n-tile width (psum free dim)
N_NT = KP // NT               # 6.5 -> 7 handled below
M_MT = KP // 128              # 13 m-tiles

_CACHED = {}


def _build_nc():
    """Raw-Bass double-buffered Gram kernel.

    All input DMAs increment ONE shared dma semaphore (order-independent
    cumulative count), so every consumer needs at most 2 sync waits —
    the hardware per-instruction wait-command limit that Tile's scheduler
    blew through for this pattern.
    """
    from contextlib import ExitStack

    import concourse.bass as bass
    import concourse.mybir as mybir

    nc = bass.Bass()
    u_dram = nc.dram_tensor("u", [HALF, KP], mybir.dt.float32, kind="ExternalInput")
    q_dram = nc.dram_tensor("q", [KP, KP], mybir.dt.float32, kind="ExternalOutput")

    n_nt = (KP + NT - 1) // NT  # 7; last n-tile is 128 wide
    m_his = [min(2 * (n + 1), M_MT) for n in range(n_nt)]
    # schedule tables: per block b -> (n, m, nt, dma count before PE may run)
    blocks = []
    din = 0
    for n in range(n_nt):
        din += KCH  # rhs strip chunks
        for m in range(m_his[n]):
            din += KCH  # lhs chunks
            blocks.append((n, m, min(NT, KP - n * NT), din))
    nblocks = len(blocks)
    cumb = np.cumsum([0] + m_his)  # blocks completed before strip n

    with ExitStack() as ctx:
        rhs_b = [
            ctx.enter_context(nc.sbuf_tensor(f"rhs{i}", [128, KCH, NT], mybir.dt.float32))
            for i in range(2)
        ]
        lhs_b = [
            ctx.enter_context(nc.sbuf_tensor(f"lhs{i}", [128, KCH, 128], mybir.dt.float32))
            for i in range(2)
        ]
        stage = [
            ctx.enter_context(nc.sbuf_tensor(f"stage{i}", [128, NT], mybir.dt.float32))
            for i in range(2)
        ]
        psum = [
            ctx.enter_context(nc.psum_tensor(f"ps{i}", [128, NT], mybir.dt.float32))
            for i in range(2)
        ]
        dma_sem = ctx.enter_context(nc.semaphore("dma_sem"))
        pe_sem = ctx.enter_context(nc.semaphore("pe_sem"))
        ve_sem = ctx.enter_context(nc.semaphore("ve_sem"))
        gp_sem = ctx.enter_context(nc.semaphore("gp_sem"))
        block = ctx.enter_context(nc.Block())

        @block.sync
        def _(sync):
            b = 0
            for n in range(n_nt):
                nt = min(NT, KP - n * NT)
                if n >= 2:  # rhs buffer reused from strip n-2
                    sync.wait_ge(pe_sem, int(cumb[n - 1]))
                for c in range(KCH):
                    sync.dma_start(
                        out=rhs_b[n % 2][:, c, :nt],
                        in_=u_dram[c * 128:(c + 1) * 128, n * NT:n * NT + nt],
                    ).then_inc(dma_sem, 16)
                for m in range(m_his[n]):
                    if b >= 2:  # lhs buffer reused from block b-2
                        sync.wait_ge(pe_sem, b - 1)
                    for c in range(KCH):
                        sync.dma_start(
                            out=lhs_b[b % 2][:, c, :],
                            in_=u_dram[c * 128:(c + 1) * 128, m * 128:(m + 1) * 128],
                        ).then_inc(dma_sem, 16)
                    b += 1

        @block.tensor
        def _(tensor):
            for b, (n, m, nt, din_b) in enumerate(blocks):
                tensor.wait_ge(dma_sem, 16 * din_b)
                if b >= 2:  # psum reused after copy of block b-2
                    tensor.wait_ge(ve_sem, b - 1)
                for c in range(KCH):
                    ins = nc.tensor.matmul(
                        psum[b % 2][:, :nt],
                        lhs_b[b % 2][:, c, :],
                        rhs_b[n % 2][:, c, :nt],
                        start=(c == 0),
                        stop=(c == KCH - 1),
                    )
                ins.then_inc(pe_sem, 1)

        @block.vector
        def _(vector):
            for b, (n, m, nt, _) in enumerate(blocks):
                vector.wait_ge(pe_sem, b + 1)
                if b >= 2:  # stage buffer reused after out-DMA of b-2
                    vector.wait_ge(gp_sem, 16 * (b - 1))
                nc.vector.tensor_copy(
                    stage[b % 2][:, :nt], psum[b % 2][:, :nt]
                ).then_inc(ve_sem, 1)

        @block.gpsimd
        def _(gpsimd):
            for b, (n, m, nt, _) in enumerate(blocks):
                gpsimd.wait_ge(ve_sem, b + 1)
                gpsimd.dma_start(
                    out=q_dram[m * 128:(m + 1) * 128, n * NT:n * NT + nt],
                    in_=stage[b % 2][:, :nt],
                ).then_inc(gp_sem, 16)

    return nc


def _unfold(x1):
    """x1: [C_in, H, W] -> U [10000, 1600] with U[(g,w'),(i,ph,pw)] = xpad[...]"""
    from numpy.lib.stride_tricks import sliding_window_view

    xp2 = np.pad(x1, ((0, 0), (4, 4), (4, 4)))
    sw = sliding_window_view(xp2, (DS, DS), axis=(1, 2))  # [C,100,100,5,5]
    return np.ascontiguousarray(
        sw.transpose(1, 2, 0, 3, 4).reshape(ROWS, K), dtype=np.float32
    )


def kernel(x, d, y, alpha, reg):
    from concourse import bass_utils

    x = np.asarray(x, dtype=np.float32)
    d = np.asarray(d, dtype=np.float32)
    y = np.asarray(y, dtype=np.float32)
    alpha = np.asarray(alpha, dtype=np.float32)
    reg = np.asarray(reg, dtype=np.float32)

    if "nc" not in _CACHED:
        _CACHED["nc"] = _build_nc()
    nc = _CACHED["nc"]

    # Host: build padded unfold matrices and shard over 8 cores.
    in_maps = []
    Us = []
    for s in range(N):
        U = _unfold(x[s, 0])  # [10000, 1600]
        Us.append(U)
        Up = np.zeros((2 * HALF, KP), dtype=np.float32)
        Up[:ROWS, :K] = U
        in_maps.append({"u": np.ascontiguousarray(Up[:HALF])})
        in_maps.append({"u": np.ascontiguousarray(Up[HALF:])})

    res = bass_utils.run_bass_kernel_spmd(nc, in_maps, core_ids=list(range(8)))
    outs = res.results

    a = alpha.reshape(N) * H * W * float(reg[0]) / (DS * DS * C_IN)

    out = np.empty((N, C_OUT, C_IN, DS, DS), dtype=np.float32)
    for s in range(N):
        Qp = outs[2 * s]["q"] + outs[2 * s + 1]["q"]
        Qu = np.triu(Qp[:K, :K].astype(np.float64))
        Q = Qu + np.triu(Qp[:K, :K].astype(np.float64), 1).T
        Q += a[s] * np.eye(K)

        # P = U^T Yz  (+ a * d): Yz is y embedded at offset (2,2) in the 100x100 grid
        Yz = np.zeros((100, 100, C_OUT), dtype=np.float32)
        Yz[2:2 + H, 2:2 + W, :] = y[s, :, 0].transpose(1, 2, 0)
        P = Us[s].T.astype(np.float64) @ Yz.reshape(ROWS, C_OUT).astype(np.float64)
        P += a[s] * d[s].transpose(1, 2, 3, 0).reshape(K, C_OUT)

        D = np.linalg.solve(Q, P)  # SPD, kappa ~ 6
        out[s] = D.reshape(C_IN, DS, DS, C_OUT).transpose(3, 0, 1, 2)
    return out



# revision 2
# speedup vs baseline: 12.6425x; 12.6425x over previous
"""Bass/Trainium2 kernel for nn_DCDicl (DSBlock forward).

Per sample: Q = Unfold_pad4(x)^T @ Unfold_pad4(x) (+ a*I), P = U^T Yz (+ a*d),
D = cho_solve(Q, P).  Q has only 64*64*81 unique values (the autocorrelation
corr[u,v,j,i] of x with itself over 81 spatial offsets); the device computes
corr and the 25-offset cross-correlation with y (= P), the host gathers the
1600x1600 Q from corr and does the small Cholesky solve in fp32.

Device sharding: data-parallel, 8 cores = 4 samples x 2 row-halves of the
padded image (contraction split).  Each core uploads ~1.7 MB and downloads
~1.4 MB, vs ~34 MB / ~11 MB for the naive unfold-Gram kernel -- the axon
link (~50 MB/s) is the bottleneck, not the device.
"""

import sys

import numpy as np

if "/opt/trn_rl_repo" not in sys.path:
    sys.path.append("/opt/trn_rl_repo")

N, C_IN, C_OUT, H, W, DS = 4, 64, 4, 96, 96, 5
K = C_IN * DS * DS            # 1600
PADH = 104                    # 96 + 2*4
BASE = PADH * PADH            # 10816 padded-grid rows
BASE_PAD = 11008              # 86 chunks of 128 (extra rows are zero)
HALF_ROWS = 5504              # 43 chunks per core
KCH = HALF_ROWS // 128        # 43
HALO = 512                    # 4 chunks of halo each side (max |shift| = 420)
SLICE_ROWS = HALF_ROWS + 2 * HALO   # 6528 = 51 chunks
G_ROWS = HALO + BASE_PAD + HALO     # 12032
NOFF = 81                     # 9x9 correlation offsets
ROWS = 100 * 100              # unfold output positions (for test.py's oracle)

_CACHED = {}


def _unfold(x1):
    """x1: [C_in, H, W] -> U [10000, 1600] (host-side; used by test.py's oracle)."""
    from numpy.lib.stride_tricks import sliding_window_view

    xp2 = np.pad(x1, ((0, 0), (4, 4), (4, 4)))
    sw = sliding_window_view(xp2, (DS, DS), axis=(1, 2))  # [C,100,100,5,5]
    return np.ascontiguousarray(
        sw.transpose(1, 2, 0, 3, 4).reshape(ROWS, K), dtype=np.float32
    )


def _is_p(o):
    u, v = o // 9, o % 9
    return 2 <= u <= 6 and 2 <= v <= 6


def _build_nc():
    """Raw-Bass kernel: per-core partial corr [81,64,64] and P [25,64,4].

    corr[o=(u,v)] = A_base^T @ A_shift(u,v); P[q=(ph,pw)] = A_shift^T @ Yt,
    where A is the [rows, 64] transposed zero-padded image slice and shifts
    are row offsets (u-4)*104 + (v-4) into the halo.
    """
    from contextlib import ExitStack

    import concourse.bass as bass
    import concourse.mybir as mybir

    nc = bass.Bass()
    a_dram = nc.dram_tensor("a", [SLICE_ROWS, C_IN], mybir.dt.float32,
                            kind="ExternalInput")
    yt_dram = nc.dram_tensor("yt", [HALF_ROWS, C_OUT], mybir.dt.float32,
                             kind="ExternalInput")
    corr_dram = nc.dram_tensor("corr", [NOFF, C_IN, C_IN], mybir.dt.float32,
                               kind="ExternalOutput")
    p_dram = nc.dram_tensor("p", [25, C_IN, C_OUT], mybir.dt.float32,
                            kind="ExternalOutput")

    # group schedule: per offset, 1 corr matmul-group (+1 P group if central)
    groups = []                     # (o, kind 'c'|'p')
    for o in range(NOFF):
        groups.append((o, "c"))
        if _is_p(o):
            groups.append((o, "p"))
    ngroups = len(groups)           # 106
    cum = np.cumsum([1 + _is_p(o) for o in range(NOFF)])  # groups thru offset o

    NPS = 4                         # psum/stage rotation depth

    with ExitStack() as ctx:
        lhs_sb = ctx.enter_context(
            nc.sbuf_tensor("lhs", [128, KCH, C_IN], mybir.dt.float32))
        yt_sb = ctx.enter_context(
            nc.sbuf_tensor("ytb", [128, KCH, C_OUT], mybir.dt.float32))
        s_sb = [ctx.enter_context(
            nc.sbuf_tensor(f"s{i}", [128, KCH, C_IN], mybir.dt.float32))
            for i in range(2)]
        stage = [ctx.enter_context(
            nc.sbuf_tensor(f"st{i}", [C_IN, C_IN], mybir.dt.float32))
            for i in range(NPS)]
        ps = [ctx.enter_context(
            nc.psum_tensor(f"ps{i}", [C_IN, C_IN], mybir.dt.float32))
            for i in range(NPS)]
        dma_sem = ctx.enter_context(nc.semaphore("dma_sem"))
        pe_sem = ctx.enter_context(nc.semaphore("pe_sem"))
        ve_sem = ctx.enter_context(nc.semaphore("ve_sem"))
        gp_sem = ctx.enter_context(nc.semaphore("gp_sem"))
        block = ctx.enter_context(nc.Block())

        @block.sync
        def _(sync):
            sync.dma_start(
                out=lhs_sb[:, :, :],
                in_=a_dram[HALO:HALO + HALF_ROWS, :].rearrange(
                    "(c p) f -> p c f", p=128),
            ).then_inc(dma_sem, 16)
            sync.dma_start(
                out=yt_sb[:, :, :],
                in_=yt_dram[:, :].rearrange("(c p) f -> p c f", p=128),
            ).then_inc(dma_sem, 16)
            for o in range(NOFF):
                off = (o // 9 - 4) * PADH + (o % 9 - 4)
                if o >= 2:  # s_sb[o%2] still consumed by offset o-2's groups
                    sync.wait_ge(pe_sem, int(cum[o - 2]))
                sync.dma_start(
                    out=s_sb[o % 2][:, :, :],
                    in_=a_dram[HALO + off:HALO + off + HALF_ROWS, :].rearrange(
                        "(c p) f -> p c f", p=128),
                ).then_inc(dma_sem, 16)

        @block.tensor
        def _(tensor):
            for g, (o, kind) in enumerate(groups):
                tensor.wait_ge(dma_sem, 16 * (2 + o + 1))
                if g >= NPS:  # psum bank reused after ve copy of g-NPS
                    tensor.wait_ge(ve_sem, g - NPS + 1)
                for c in range(KCH):
                    if kind == "c":
                        ins = nc.tensor.matmul(
                            ps[g % NPS][:, :],
                            lhs_sb[:, c, :],
                            s_sb[o % 2][:, c, :],
                            start=(c == 0),
                            stop=(c == KCH - 1),
                        )
                    else:
                        ins = nc.tensor.matmul(
                            ps[g % NPS][:, :C_OUT],
                            s_sb[o % 2][:, c, :],
                            yt_sb[:, c, :],
                            start=(c == 0),
                            stop=(c == KCH - 1),
                        )
                ins.then_inc(pe_sem, 1)

        @block.vector
        def _(vector):
            for g, (o, kind) in enumerate(groups):
                vector.wait_ge(pe_sem, g + 1)
                if g >= NPS:  # stage buffer reused after out-DMA of g-NPS
                    vector.wait_ge(gp_sem, 16 * (g - NPS + 1))
                w = C_IN if kind == "c" else C_OUT
                nc.vector.tensor_copy(
                    stage[g % NPS][:, :w], ps[g % NPS][:, :w]
                ).then_inc(ve_sem, 1)

        @block.gpsimd
        def _(gpsimd):
            for g, (o, kind) in enumerate(groups):
                gpsimd.wait_ge(ve_sem, g + 1)
                if kind == "c":
                    gpsimd.dma_start(
                        out=corr_dram[o], in_=stage[g % NPS][:, :]
                    ).then_inc(gp_sem, 16)
                else:
                    u, v = o // 9, o % 9
                    q = (u - 2) * 5 + (v - 2)
                    gpsimd.dma_start(
                        out=p_dram[q], in_=stage[g % NPS][:, :C_OUT]
                    ).then_inc(gp_sem, 16)

    return nc


def _get_runner():
    """Cached jitted executor: (concat_a, concat_yt) -> (corr_g, p_g)."""
    if "runner" in _CACHED:
        return _CACHED["runner"]

    import jax
    from jax.experimental.shard_map import shard_map
    from jax.sharding import Mesh, PartitionSpec

    from concourse import bass2jax
    from concourse.bass2jax import _bass_exec_p, install_neuronx_cc_hook
    import concourse.mybir as mybir

    install_neuronx_cc_hook()
    nc = _build_nc()

    partition_name = (
        nc.partition_id_tensor.name if nc.partition_id_tensor else None
    )
    in_names = []
    out_names = []
    out_avals = []
    for alloc in nc.m.functions[0].allocations:
        if not isinstance(alloc, mybir.MemoryLocationSet):
            continue
        name = alloc.memorylocations[0].name
        if alloc.kind == "ExternalInput":
            if name != partition_name:
                in_names.append(name)
        elif alloc.kind == "ExternalOutput":
            out_names.append(name)
            out_avals.append(jax.core.ShapedArray(
                tuple(alloc.tensor_shape), mybir.dt.np(alloc.dtype)))
    n_params = len(in_names)
    if partition_name is not None:
        in_names.append(partition_name)

    def _body(*args):
        operands = list(args)
        if partition_name is not None:
            operands.append(bass2jax.partition_id_tensor())
        outs = _bass_exec_p.bind(
            *operands,
            out_avals=tuple(out_avals),
            in_names=tuple(in_names),
            out_names=tuple(out_names),
            lowering_input_output_aliases=(),
            sim_require_finite=True,
            sim_require_nnan=True,
            nc=nc,
        )
        return tuple(outs)

    devices = jax.devices()[:8]
    mesh = Mesh(np.asarray(devices), ("core",))
    sharded = jax.jit(
        shard_map(
            _body, mesh=mesh,
            in_specs=(PartitionSpec("core"),) * n_params,
            out_specs=(PartitionSpec("core"),) * len(out_names),
            check_rep=False,
        ),
        keep_unused=True,
    )
    _CACHED["runner"] = sharded
    return sharded


def kernel(x, d, y, alpha, reg):
    x = np.asarray(x, dtype=np.float32)
    d = np.asarray(d, dtype=np.float32)
    y = np.asarray(y, dtype=np.float32)
    alpha = np.asarray(alpha, dtype=np.float32)
    reg = np.asarray(reg, dtype=np.float32)

    runner = _get_runner()

    # --- host prep: per-core transposed padded-image slices ---
    xp = np.pad(x[:, 0], ((0, 0), (0, 0), (4, 4), (4, 4)))   # [4,64,104,104]
    G = np.zeros((N, G_ROWS, C_IN), dtype=np.float32)
    G[:, HALO:HALO + BASE] = xp.reshape(N, C_IN, BASE).transpose(0, 2, 1)
    yp = np.pad(y[:, :, 0], ((0, 0), (0, 0), (4, 4), (4, 4)))  # [4,4,104,104]
    Yg = np.zeros((N, BASE_PAD, C_OUT), dtype=np.float32)
    Yg[:, :BASE] = yp.reshape(N, C_OUT, BASE).transpose(0, 2, 1)

    concat_a = np.empty((2 * N, SLICE_ROWS, C_IN), dtype=np.float32)
    concat_yt = np.empty((2 * N, HALF_ROWS, C_OUT), dtype=np.float32)
    for s in range(N):
        for c in range(2):
            concat_a[2 * s + c] = G[s, HALF_ROWS * c:HALF_ROWS * c + SLICE_ROWS]
            concat_yt[2 * s + c] = Yg[s, HALF_ROWS * c:HALF_ROWS * c + HALF_ROWS]

    corr_g, p_g = runner(
        concat_a.reshape(-1, C_IN), concat_yt.reshape(-1, C_OUT))
    corr_parts = np.asarray(corr_g).reshape(2 * N, NOFF, C_IN, C_IN)
    p_parts = np.asarray(p_g).reshape(2 * N, 25, C_IN, C_OUT)

    # --- host post: sum halves, gather Q, solve ---
    from numpy.lib.stride_tricks import sliding_window_view
    import scipy.linalg as sla

    a = alpha.reshape(N) * H * W * float(reg[0]) / (DS * DS * C_IN)

    corr = (corr_parts[0::2] + corr_parts[1::2]).reshape(N, 9, 9, C_IN, C_IN)
    corr[:, 4, 4, np.arange(C_IN), np.arange(C_IN)] += a[:, None]
    Pm = (p_parts[0::2] + p_parts[1::2]).transpose(0, 2, 1, 3).reshape(N, K, C_OUT)
    Pm = Pm + a[:, None, None] * d.transpose(0, 2, 3, 4, 1).reshape(N, K, C_OUT)

    # Q[(j,kh,kw),(i,ph,pw)] = corr[ph-kh+4, pw-kw+4, j, i]
    sw = sliding_window_view(corr, (DS, DS), axis=(1, 2))  # [s,5,5,j,i,ph,pw]
    Qs = sw[:, ::-1, ::-1]                                 # index by (kh,kw)
    Q = np.ascontiguousarray(
        Qs.transpose(0, 3, 1, 2, 4, 5, 6)                  # [s,j,kh,kw,i,ph,pw]
    ).reshape(N, K, K)

    out = np.empty((N, C_OUT, C_IN, DS, DS), dtype=np.float32)
    for s in range(N):
        cf = sla.cho_factor(Q[s], lower=True, check_finite=False)
        D = sla.cho_solve(cf, Pm[s], check_finite=False)
        out[s] = D.reshape(C_IN, DS, DS, C_OUT).transpose(3, 0, 1, 2)
    return out
